# revision 13
# baseline (speedup 1.0000x reference)
"""Trainium2 Bass kernel for a DoReFa-quantized ResNet BasicBlock (inference).

Reference computation (all fp32):
    out = qact(bn2(conv3x3(qact(bn1(conv3x3(x, qw(w1)))), qw(w2))) + x)
with qw = 4-bit DoReFa weight quant, qact = 4-bit activation quant,
x: (64, 128, 56, 56), convs 128->128 stride 1 pad 1.

Sharding: data-parallel over the batch dim, 8 images per NeuronCore on 8 cores.

Per-core kernel design:
  * NCHW with C=128 on SBUF partitions, flattened zero-padded image rows in
    the free dim; a 3x3 conv = shifted 128x128 matmuls accumulated in PSUM
    (8-row chunks, one PSUM bank each).
  * Dual row pitches: conv1/x use 58-wide rows; act1/conv2 use 64-wide rows
    so the fp8 DoubleRow pair stride (2*64 = 128 B) satisfies the %16 rule.
  * Conv2 is bit-exact integer arithmetic in 5 fp8 matmuls per chunk
    (3 DoubleRow dy-pairs, 1 DoubleRow (dy=0,dx=-1)+(dx=+1) pair against a
    16-byte-aligned shifted duplicate of act1, 1 normal center tap).
  * Conv1 streams x in the PE's float32r mode at 1 col/cycle; its stationary
    weights are exact small ints, stored fp8 so LDWEIGHTS (~95ns) hides
    under the 187ns stream (f32r weights load in ~186ns and cap the MM
    cadence at ~207ns).
  * Tap-major matmul emission over chunk groups (conv1 {0..3}/{4..6}, conv2
    {0..2}/{3..6}); 4+4 PSUM banks.
  * qact is split into row halves emitted right after each conv1 group so
    the final image's conv2 is never gated on a full-image DVE chain.
  * Pad borders in the x and act1 tiles are written only for the first ring
    generation of each tile slot (all interior writes are border-disjoint),
    removing ~12 GpSimd memsets per image.
  * BN folds to a per-channel affine applied by ScalarE out of PSUM; DoReFa
    staircase = tensor_scalar clip (max,min) + round-half-even via the +2^23
    fp32 trick on VectorE (bit-matches jnp.round).
  * All HBM transfers contiguous (staging tiles + VectorE pad insert); image
    0 is chunk-major, paced by its quarter DMAs, with w1 tap-block DMAs
    interleaved behind the x quarters.
  * A post-Tile pass splits multi-semaphore waits onto same-engine NoOps
    (this walrus encodes at most one sync wait per instruction).
"""

import os
import sys

import numpy as np

for _p in ("/opt/trn_rl_repo", "/opt/pypackages"):
    if _p not in sys.path and os.path.isdir(_p):
        sys.path.insert(0, _p)

import ml_dtypes  # noqa: E402

# ---------------------------------------------------------------- constants
B, C, H, W = 64, 128, 56, 56
N_CORES = 8
BPC = B // N_CORES          # images per core
WP = W + 2                  # conv1/x padded row length (58)
WP2 = 64                    # conv2/act1 padded row length (58 used + 6 dead)
HPAD = H + 2                # padded rows        (58)
IMG = WP * HPAD             # x-layout padded image elems (3364)
IMG2 = WP2 * HPAD           # act1-layout padded image elems (3712)
BUF = IMG + 4               # x/v1 buffer
BUF2 = IMG2 + 4             # act1/v2 buffer
ACT_D = 3726                # shifted act1 copy offset; pair step D+2 %16==0
ABUF = ACT_D + BUF2         # act1 tile width (original + shifted copy)
XB = 1                      # x / v / out buffers: image base offset
AB = 2                      # act1 buffer: base offset
RPC = 8                     # padded rows per PSUM chunk
NCHUNK = H // RPC           # 7 chunks cover output rows 1..56
FREE = RPC * W              # 448 free elems per conv matmul (56-wide)
MAGIC = float(2**23)        # fp32 round-to-nearest-even magic constant
EPS = 1e-5

G1 = ((0, 1, 2, 3), (4, 5, 6))   # conv1 tap-major chunk groups
G2 = ((0, 1), (2, 3), (4, 5), (6,))   # conv2 sub-groups
QUARTERS = ((0,), (1,), (2,), (3,))    # one output quarter after each

# conv1 stationary-weight storage: "f8" | "bf16" | "f32r" (values are exact
# odd ints in [-15,15] in all three; narrower dtypes load faster)
W1DT = os.environ.get("K_W1DT", "f8")
XDT = os.environ.get("K_XDT", "f32r")
if XDT == "f32r":
    W1DT = "f32r"

_CACHE = {}


# ---------------------------------------------------------------- host math
def _quant_weight_int(w):
    """Return 15*quantize_weight(w, 4) which is an exact odd integer in
    [-15, 15], as float32."""
    wt = np.tanh(w.astype(np.float64)).astype(np.float32)
    m = np.float32(np.abs(wt).max())
    wtn = wt / (np.float32(2.0) * m) + np.float32(0.5)      # [0, 1]
    q = np.round(wtn * np.float32(15.0)).astype(np.float32)  # {0..15}, half-even
    return np.float32(2.0) * q - np.float32(15.0)            # odd ints [-15,15]


def _bn_affine(gamma, beta, mean, var):
    """Per-channel (scale, bias) with bn(y) = scale*y + bias, in f64."""
    inv = 1.0 / np.sqrt(var.astype(np.float64) + EPS)
    s = gamma.astype(np.float64) * inv
    b = beta.astype(np.float64) - mean.astype(np.float64) * s
    return s, b


def _lhsT_taps(w_int):
    """[oc, ic, 3, 3] -> [ic, 9*oc] stationary-operand layout (tap-major)."""
    t = np.transpose(w_int, (2, 3, 1, 0)).reshape(9, C, C)   # [tap, ic, oc]
    return np.transpose(t, (1, 0, 2)).reshape(C, 9 * C)


# ---------------------------------------------------------------- bass build
def _split_multiwaits(nc, mybir):
    """Walrus in this toolchain encodes at most ONE sync wait per instruction.

    Tile emits instructions with several on_wait entries; hoist all but one
    onto same-engine NoOps placed immediately before the instruction."""
    nid = 0
    for fn in nc.m.functions:
        for blk in fn.blocks:
            out = []
            changed = False
            for ins in blk.instructions:
                si = ins.sync_info
                if si is not None and len(si.on_wait) > 1:
                    waits = list(si.on_wait)
                    for w in waits[:-1]:
                        nid += 1
                        nop = mybir.InstNoOp(name=f"I-wfix-{nid}",
                                             engine=ins.engine)
                        nop.sync_info = mybir.SyncInfo(on_wait=[w],
                                                       on_update=[])
                        out.append(nop)
                    ins.sync_info = mybir.SyncInfo(
                        on_wait=[waits[-1]], on_update=list(si.on_update))
                    changed = True
                out.append(ins)
            if changed:
                blk.instructions = out


def _build_module(apply_wfix=True):
    import concourse.bass as bass
    import concourse.mybir as mybir
    import concourse.tile as tile
    from contextlib import ExitStack

    f32 = mybir.dt.float32
    AF = mybir.ActivationFunctionType
    OP = mybir.AluOpType

    nc = bass.Bass("TRN2", target_bir_lowering=False, debug=False,
                   num_devices=N_CORES)

    f32r = mybir.dt.float32r
    f8 = mybir.dt.float8e4
    w1dt = {"f8": f8, "bf16": mybir.dt.bfloat16, "f32r": f32r}[W1DT]

    f16 = mybir.dt.float16
    xdt = f16 if XDT == "f16" else f32r
    x_d = nc.dram_tensor("x15", [BPC, C, H, W], xdt, kind="ExternalInput")
    w2p_d = nc.dram_tensor("w2p", [C, 9 * C], f8, kind="ExternalInput")
    w1r_d = nc.dram_tensor("w1r", [C, 9 * C], w1dt, kind="ExternalInput")
    # columns: [sc1, bi1, sc2, bi2]
    bn_d = nc.dram_tensor("bnv", [C, 4], f32, kind="ExternalInput")
    out_d = nc.dram_tensor("out", [BPC, C, H, W], f32, kind="ExternalOutput")

    with tile.TileContext(nc) as tc, ExitStack() as ctx:
        const = ctx.enter_context(tc.tile_pool(name="const", bufs=1))
        sb = ctx.enter_context(tc.tile_pool(name="sb", bufs=2))
        xp = ctx.enter_context(tc.tile_pool(name="xp", bufs=3))
        ps = ctx.enter_context(tc.tile_pool(name="ps", bufs=4, space="PSUM"))

        warm_w = const.tile([C, C], f32r)
        warm_x = const.tile([C, FREE], f32r)
        w1r_sb = const.tile([C, 9 * C], w1dt)
        bn_sb = const.tile([C, 4], f32)
        w2p_sb = const.tile([C, 9 * C], f8)
        sc1_sb = bn_sb[:, 0:1]
        bi1_sb = bn_sb[:, 1:2]
        sc2_sb = bn_sb[:, 2:3]
        bi2_sb = bn_sb[:, 3:4]

        def emit_x_load(n, first=False):
            """Allocate x tile for image n, DMA + pad-insert (borders are
            memset only for the first ring generation of each slot; interior
            writes never touch them)."""
            x = xp.tile([C, BUF], f16 if XDT == "f16" else f32,
                        tag="x", name=f"x_{n}")
            xw = x if XDT == "f16" else x.bitcast(f32r)
            xwr = xw[:, XB:XB + IMG].rearrange("p (h w) -> p h w", w=WP)
            if n < 3:
                xr0 = x[:, XB:XB + IMG].rearrange("p (h w) -> p h w", w=WP)
                nc.gpsimd.memset(x[:, 0:XB + WP + 1], 0.0)
                nc.gpsimd.memset(x[:, XB + (HPAD - 1) * WP:BUF], 0.0)
                nc.gpsimd.memset(xr0[:, 1:57, 0], 0.0)
                nc.gpsimd.memset(xr0[:, 1:57, 57], 0.0)
            if first:
                # const loads ride the Scalar engine's DGE ring so they
                # stream in parallel with the x pieces on the Sync ring
                for b in range(3):
                    nc.scalar.dma_start(
                        w1r_sb[:, b * 3 * C:(b + 1) * 3 * C],
                        w1r_d.ap()[:, b * 3 * C:(b + 1) * 3 * C])
                nc.scalar.dma_start(bn_sb[:], bn_d.ap())
                nc.scalar.dma_start(w2p_sb[:], w2p_d.ap())
            xs = sb.tile([C, H * W], xdt, tag="xs", name=f"xs_{n}")
            xsr = xs.rearrange("p (h w) -> p h w", w=W)
            xd_flat = x_d.ap()[n].rearrange("p h w -> p (h w)")
            npiece = 8 if first else 4
            rows = H // npiece
            for q in range(npiece):
                r0, r1 = 1 + rows * q, 1 + rows * (q + 1)
                nc.sync.dma_start(xs[:, (r0 - 1) * W:(r1 - 1) * W],
                                  xd_flat[:, (r0 - 1) * W:(r1 - 1) * W])
                nc.vector.tensor_copy(
                    xwr[:, r0:r1, 1:57],
                    xsr[:, r0 - 1:r1 - 1, :])
            return x

        def conv1_rhs(x, cch, dy, dx):
            r0 = 1 + RPC * cch
            off = XB + (r0 + dy) * WP + 1 + dx
            mv = bass.AP(tensor=x.tensor, offset=off,
                         ap=[[BUF, C], [WP, RPC], [1, W]])
            return mv if XDT == "f16" else mv.bitcast(f32r)

        def conv1_affine(v1, cch, p1):
            r0 = 1 + RPC * cch
            dst = v1[:, XB + r0 * WP:XB + (r0 + RPC) * WP].rearrange(
                "p (h w) -> p h w", w=WP)[:, :, 1:57]
            nc.scalar.activation(dst,
                                 p1.rearrange("p (h w) -> p h w", w=W),
                                 AF.Identity, bias=bi1_sb, scale=sc1_sb)

        def emit_conv1_group(n, x, v1, chunks, tap_major=True):
            """Conv1 matmuls + affines over `chunks`.  Tap-major shares the
            stationary operand across the group; chunk-major (image 0) lets
            each chunk start as soon as its x quarter lands."""
            p1s = [ps.tile([C, FREE], f32, tag="p1", name=f"p1_{n}_{c}")
                   for c in chunks]
            order = [(t9, ci) for t9 in range(9) for ci in range(len(chunks))]
            if not tap_major:
                order = [(t9, ci) for ci in range(len(chunks))
                         for t9 in range(9)]
            for t9, ci in order:
                dy, dx = t9 // 3 - 1, t9 % 3 - 1
                nc.tensor.matmul(
                    p1s[ci][:],
                    lhsT=w1r_sb[:, t9 * C:(t9 + 1) * C],
                    rhs=conv1_rhs(x, chunks[ci], dy, dx),
                    start=(t9 == 0), stop=(t9 == 8))
            for ci, cch in enumerate(chunks):
                conv1_affine(v1, cch, p1s[ci])

        def emit_qact(n, v1, act1, half):
            """clip + round for v1 rows ra..rb -> act1 fp8 (both copies)."""
            ra, rb = (1, 33) if half == 0 else (33, 57)
            v1r = v1[:, XB:XB + IMG].rearrange("p (h w) -> p h w", w=WP)
            nc.vector.tensor_scalar(v1r[:, ra:rb, 1:57], v1r[:, ra:rb, 1:57],
                                    0.0, 15.0, op0=OP.max, op1=OP.min)
            ar = act1[:, AB:AB + IMG2].rearrange("p (h w) -> p h w", w=WP2)
            nc.vector.tensor_scalar(ar[:, ra:rb, 1:57], v1r[:, ra:rb, 1:57],
                                    MAGIC, MAGIC, op0=OP.add, op1=OP.subtract)
            ar2 = act1[:, ACT_D + AB:ACT_D + AB + IMG2].rearrange(
                "p (h w) -> p h w", w=WP2)
            nc.vector.tensor_scalar(ar2[:, ra:rb, 1:57], v1r[:, ra:rb, 1:57],
                                    MAGIC, MAGIC, op0=OP.add, op1=OP.subtract)

        def new_act1(n):
            """Allocate the act1 tile; zero its static border region for the
            first ring generation of each slot."""
            act1 = sb.tile([C, ABUF], f8, tag="act1", name=f"act1_{n}")
            if n < 2:
                ar = act1[:, AB:AB + IMG2].rearrange(
                    "p (h w) -> p h w", w=WP2)
                ar2 = act1[:, ACT_D + AB:ACT_D + AB + IMG2].rearrange(
                    "p (h w) -> p h w", w=WP2)
                nc.gpsimd.memset(act1[:, 0:AB + WP2 + 1], 0.0)
                nc.gpsimd.memset(act1[:, AB + (HPAD - 1) * WP2:BUF2], 0.0)
                nc.gpsimd.memset(ar[:, 1:57, 0], 0.0)
                nc.gpsimd.memset(ar[:, 1:57, 57:64], 0.0)
                nc.gpsimd.memset(act1[:, ACT_D:ACT_D + AB + WP2 + 1], 0.0)
                nc.gpsimd.memset(act1[:, ACT_D + AB + (HPAD - 1) * WP2:ABUF],
                                 0.0)
                nc.gpsimd.memset(ar2[:, 1:57, 0], 0.0)
                nc.gpsimd.memset(ar2[:, 1:57, 57:64], 0.0)
            return act1

        def emit_conv2_group(n, x, act1, v2, ost, chunks, quarters):
            """Weight-set-major conv2 over `chunks`, then residual+qact
            quarters (56-wide streams throughout)."""
            inv15 = float(np.float32(1.0) / np.float32(15.0))
            vr = v2[:, XB:XB + IMG2].rearrange("p (h w) -> p h w", w=WP2)
            xr = x[:, XB:XB + IMG].rearrange("p (h w) -> p h w", w=WP)
            ostr = ost.rearrange("p (h w) -> p h w", w=W)
            od_flat = out_d.ap()[n].rearrange("p h w -> p (h w)")

            p2s = [ps.tile([C, FREE], f32, tag="p2", name=f"p2_{n}_{c}")
                   for c in chunks]
            for mi in range(5):
                for ci, cch in enumerate(chunks):
                    r0 = 1 + RPC * cch
                    if mi < 3:
                        dx = mi - 1
                        off_a = AB + (r0 - 1) * WP2 + 1 + dx
                        mv = bass.AP(tensor=act1.tensor, offset=off_a,
                                     ap=[[ABUF, C], [2 * WP2, 2], [WP2, RPC],
                                         [1, W]])
                        wpair = w2p_sb[:, mi * 2 * C:(mi + 1) * 2 * C]\
                            .rearrange("p (two m) -> p two m", two=2)
                        nc.tensor.matmul(
                            p2s[ci][:], lhsT=wpair, rhs=mv,
                            perf_mode=mybir.MatmulPerfMode.DoubleRow,
                            start=(mi == 0), stop=False)
                    elif mi == 3:
                        off_a = AB + r0 * WP2
                        mv = bass.AP(tensor=act1.tensor, offset=off_a,
                                     ap=[[ABUF, C], [ACT_D + 2, 2],
                                         [WP2, RPC], [1, W]])
                        wpair = w2p_sb[:, 6 * C:8 * C].rearrange(
                            "p (two m) -> p two m", two=2)
                        nc.tensor.matmul(
                            p2s[ci][:], lhsT=wpair, rhs=mv,
                            perf_mode=mybir.MatmulPerfMode.DoubleRow,
                            start=False, stop=False)
                    else:
                        off = AB + r0 * WP2 + 1
                        mv = bass.AP(tensor=act1.tensor, offset=off,
                                     ap=[[ABUF, C], [WP2, RPC], [1, W]])
                        nc.tensor.matmul(p2s[ci][:],
                                         lhsT=w2p_sb[:, 8 * C:9 * C],
                                         rhs=mv, start=False, stop=True)
            for ci, cch in enumerate(chunks):
                r0 = 1 + RPC * cch
                dst = v2[:, XB + r0 * WP2:XB + (r0 + RPC) * WP2].rearrange(
                    "p (h w) -> p h w", w=WP2)[:, :, 1:57]
                nc.scalar.activation(dst,
                                     p2s[ci].rearrange("p (h w) -> p h w",
                                                       w=W),
                                     AF.Identity, bias=bi2_sb, scale=sc2_sb)

            for q in quarters:
                # residual + qact + /15 + out-DMA for output rows 14q..14q+13,
                # in 7-row pieces so the gpsimd/scalar/DVE chain pipelines
                for hh in range(2):
                    r0, r1 = 1 + 14 * q + 7 * hh, 8 + 14 * q + 7 * hh
                    vq = vr[:, r0:r1, 1:57]
                    nc.gpsimd.tensor_add(vq, vq, xr[:, r0:r1, 1:57])
                    nc.scalar.activation(vq, vq, AF.Relu)
                    nc.vector.tensor_scalar(vq, vq, 15.0, MAGIC,
                                            op0=OP.min, op1=OP.add)
                    nc.vector.tensor_scalar(ostr[:, r0 - 1:r1 - 1, :],
                                            vr[:, r0:r1, 1:57], MAGIC, inv15,
                                            op0=OP.subtract, op1=OP.mult)
                    nc.sync.dma_start(od_flat[:, (r0 - 1) * W:(r1 - 1) * W],
                                      ost[:, (r0 - 1) * W:(r1 - 1) * W])

        # ------------------------------------------------------- pipeline
        NWARM = int(os.environ.get("K_NWARM", "6"))
        if NWARM:
            nc.gpsimd.memset(warm_w.bitcast(f32)[:], 1.0)
            nc.gpsimd.memset(warm_x.bitcast(f32)[:], 1.0)
            pwm = ps.tile([C, FREE], f32, tag="p1", name="p_warm")
            for _ in range(NWARM):
                nc.tensor.matmul(pwm[:], lhsT=warm_w[:], rhs=warm_x[:],
                                 start=True, stop=True)
        img_v2 = {}

        def conv2_part(n, x_im, act1_im, gi):
            if n not in img_v2:
                img_v2[n] = (sb.tile([C, BUF2], f32, tag="v2",
                                     name=f"v2_{n}"),
                             sb.tile([C, H * W], f32, tag="ost",
                                     name=f"ost_{n}"))
            v2, ost = img_v2[n]
            emit_conv2_group(n, x_im, act1_im, v2, ost, G2[gi], QUARTERS[gi])

        x0 = emit_x_load(0, first=True)
        v1_0 = sb.tile([C, BUF], f32, tag="v1", name="v1_0")
        act1_0 = new_act1(0)
        emit_conv1_group(0, x0, v1_0, (0, 1))
        emit_conv1_group(0, x0, v1_0, (2, 3))
        emit_qact(0, v1_0, act1_0, 0)
        emit_conv1_group(0, x0, v1_0, (4, 5, 6))
        emit_qact(0, v1_0, act1_0, 1)
        conv2_part(0, x0, act1_0, 0)
        conv2_part(0, x0, act1_0, 1)
        conv2_part(0, x0, act1_0, 2)
        x_cur, act1_cur = x0, act1_0

        for n in range(BPC):
            x_nxt = v1_nxt = act1_nxt = None
            if n + 1 < BPC:
                x_nxt = emit_x_load(n + 1)
                v1_nxt = sb.tile([C, BUF], f32, tag="v1", name=f"v1_{n + 1}")
                act1_nxt = new_act1(n + 1)
                emit_conv1_group(n + 1, x_nxt, v1_nxt, G1[0])
                emit_qact(n + 1, v1_nxt, act1_nxt, 0)

            conv2_part(n, x_cur, act1_cur, 3)

            if n + 1 < BPC:
                emit_conv1_group(n + 1, x_nxt, v1_nxt, G1[1])
                emit_qact(n + 1, v1_nxt, act1_nxt, 1)
                # pull image n+1's conv2 groups 0-2 into this round so the
                # final round only drains one group + one quarter chain
                conv2_part(n + 1, x_nxt, act1_nxt, 0)
                conv2_part(n + 1, x_nxt, act1_nxt, 1)
                conv2_part(n + 1, x_nxt, act1_nxt, 2)

            x_cur, act1_cur = x_nxt, act1_nxt

    if apply_wfix:
        _split_multiwaits(nc, mybir)
    return nc


def _get_module(apply_wfix=True):
    key = ("nc", apply_wfix, W1DT, XDT)
    if key not in _CACHE:
        _CACHE[key] = _build_module(apply_wfix)
    return _CACHE[key]


# ---------------------------------------------------------------- host entry
def _make_in_maps(x, w1, w2, gamma1, beta1, mean1, var1,
                  gamma2, beta2, mean2, var2):
    x15 = np.float32(15.0) * np.asarray(x, np.float32)
    if XDT == "f16":
        x15 = x15.astype(np.float16)
    x15 = x15.reshape(N_CORES, BPC, C, H, W)

    w1i = _quant_weight_int(np.asarray(w1, np.float32))
    w2i = _quant_weight_int(np.asarray(w2, np.float32))
    w2t = _lhsT_taps(w2i)  # [C, 9*C], tap-major (t9 = (dy+1)*3 + dx+1)
    tap = lambda t9: w2t[:, t9 * C:(t9 + 1) * C]
    blocks = []
    for dxi in range(3):           # DR pairs: (dy=-1,dx) then (dy=+1,dx)
        blocks += [tap(dxi), tap(6 + dxi)]
    blocks += [tap(3), tap(5)]     # DR pair: (dy=0,dx=-1) + (dy=0,dx=+1)
    blocks.append(tap(4))          # single: (dy=0,dx=0)
    w2p = np.concatenate(blocks, axis=1).astype(ml_dtypes.float8_e4m3)

    s1, b1 = _bn_affine(np.asarray(gamma1, np.float32), np.asarray(beta1, np.float32),
                        np.asarray(mean1, np.float32), np.asarray(var1, np.float32))
    s2, b2 = _bn_affine(np.asarray(gamma2, np.float32), np.asarray(beta2, np.float32),
                        np.asarray(mean2, np.float32), np.asarray(var2, np.float32))
    # conv PSUM holds 225*conv (15x-or-15a input, 15w weights) -> want 15*bn:
    bnv = np.stack([s1 / 15.0, 15.0 * b1, s2 / 15.0, 15.0 * b2],
                   axis=1).astype(np.float32)  # [C, 4]

    w1np = {"f8": ml_dtypes.float8_e4m3, "bf16": ml_dtypes.bfloat16,
            "f32r": np.float32}[W1DT]
    w1r = _lhsT_taps(w1i).astype(w1np)
    shared = {"w2p": w2p, "w1r": w1r, "bnv": bnv}
    return [{"x15": np.ascontiguousarray(x15[i]), **shared}
            for i in range(N_CORES)]


def kernel(**inputs):
    from concourse.bass_utils import run_bass_kernel_spmd

    nc = _get_module()
    in_maps = _make_in_maps(**inputs)
    res = run_bass_kernel_spmd(nc, in_maps, core_ids=list(range(N_CORES)))
    _CACHE["last_res"] = res
    out = np.concatenate([np.asarray(r["out"], np.float32)
                          for r in res.results], axis=0)
    return out.reshape(B, C, H, W)


# revision 14
# speedup vs baseline: 1.0076x; 1.0076x over previous
"""Trainium2 Bass kernel for a DoReFa-quantized ResNet BasicBlock (inference).

Reference computation (all fp32):
    out = qact(bn2(conv3x3(qact(bn1(conv3x3(x, qw(w1)))), qw(w2))) + x)
with qw = 4-bit DoReFa weight quant, qact = 4-bit activation quant,
x: (64, 128, 56, 56), convs 128->128 stride 1 pad 1.

Sharding: data-parallel over the batch dim, 8 images per NeuronCore on 8 cores.

Per-core kernel design:
  * NCHW with C=128 on SBUF partitions, flattened zero-padded image rows in
    the free dim; a 3x3 conv = shifted 128x128 matmuls accumulated in PSUM
    (8-row chunks, one PSUM bank each).
  * Dual row pitches: conv1/x use 58-wide rows; act1/conv2 use 64-wide rows
    so the fp8 DoubleRow pair stride (2*64 = 128 B) satisfies the %16 rule.
  * Conv2 is bit-exact integer arithmetic in 5 fp8 matmuls per chunk
    (3 DoubleRow dy-pairs, 1 DoubleRow (dy=0,dx=-1)+(dx=+1) pair against a
    16-byte-aligned shifted duplicate of act1, 1 normal center tap).
  * Conv1 streams x in the PE's float32r mode at 1 col/cycle; its stationary
    weights are exact small ints, stored fp8 so LDWEIGHTS (~95ns) hides
    under the 187ns stream (f32r weights load in ~186ns and cap the MM
    cadence at ~207ns).
  * Tap-major matmul emission over chunk groups (conv1 {0..3}/{4..6}, conv2
    {0..2}/{3..6}); 4+4 PSUM banks.
  * qact is split into row halves emitted right after each conv1 group so
    the final image's conv2 is never gated on a full-image DVE chain.
  * Pad borders in the x and act1 tiles are written only for the first ring
    generation of each tile slot (all interior writes are border-disjoint),
    removing ~12 GpSimd memsets per image.
  * BN folds to a per-channel affine applied by ScalarE out of PSUM; DoReFa
    staircase = tensor_scalar clip (max,min) + round-half-even via the +2^23
    fp32 trick on VectorE (bit-matches jnp.round).
  * All HBM transfers contiguous (staging tiles + VectorE pad insert); image
    0 is chunk-major, paced by its quarter DMAs, with w1 tap-block DMAs
    interleaved behind the x quarters.
  * A post-Tile pass splits multi-semaphore waits onto same-engine NoOps
    (this walrus encodes at most one sync wait per instruction).
"""

import os
import sys

import numpy as np

for _p in ("/opt/trn_rl_repo", "/opt/pypackages"):
    if _p not in sys.path and os.path.isdir(_p):
        sys.path.insert(0, _p)

import ml_dtypes  # noqa: E402

# ---------------------------------------------------------------- constants
B, C, H, W = 64, 128, 56, 56
N_CORES = 8
BPC = B // N_CORES          # images per core
WP = W + 2                  # conv1/x padded row length (58)
WP2 = 64                    # conv2/act1 padded row length (58 used + 6 dead)
HPAD = H + 2                # padded rows        (58)
IMG = WP * HPAD             # x-layout padded image elems (3364)
IMG2 = WP2 * HPAD           # act1-layout padded image elems (3712)
BUF = IMG + 4               # x/v1 buffer
BUF2 = IMG2 + 4             # act1/v2 buffer
ACT_D = 3726                # shifted act1 copy offset; pair step D+2 %16==0
ABUF = ACT_D + BUF2         # act1 tile width (original + shifted copy)
XB = 1                      # x / v / out buffers: image base offset
AB = 2                      # act1 buffer: base offset
RPC = 8                     # padded rows per PSUM chunk
NCHUNK = H // RPC           # 7 chunks cover output rows 1..56
FREE = RPC * W              # 448 free elems per conv matmul (56-wide)
MAGIC = float(2**23)        # fp32 round-to-nearest-even magic constant
EPS = 1e-5

G1 = ((0, 1, 2, 3), (4, 5, 6))   # conv1 tap-major chunk groups
G2 = ((0, 1), (2, 3), (4, 5), (6,))   # conv2 sub-groups
QUARTERS = ((0,), (1,), (2,), (3,))    # one output quarter after each

# conv1 stationary-weight storage: "f8" | "bf16" | "f32r" (values are exact
# odd ints in [-15,15] in all three; narrower dtypes load faster)
W1DT = os.environ.get("K_W1DT", "f8")
XDT = os.environ.get("K_XDT", "f32r")
if XDT == "f32r":
    W1DT = "f32r"

_CACHE = {}


# ---------------------------------------------------------------- host math
def _quant_weight_int(w):
    """Return 15*quantize_weight(w, 4) which is an exact odd integer in
    [-15, 15], as float32."""
    wt = np.tanh(w.astype(np.float64)).astype(np.float32)
    m = np.float32(np.abs(wt).max())
    wtn = wt / (np.float32(2.0) * m) + np.float32(0.5)      # [0, 1]
    q = np.round(wtn * np.float32(15.0)).astype(np.float32)  # {0..15}, half-even
    return np.float32(2.0) * q - np.float32(15.0)            # odd ints [-15,15]


def _bn_affine(gamma, beta, mean, var):
    """Per-channel (scale, bias) with bn(y) = scale*y + bias, in f64."""
    inv = 1.0 / np.sqrt(var.astype(np.float64) + EPS)
    s = gamma.astype(np.float64) * inv
    b = beta.astype(np.float64) - mean.astype(np.float64) * s
    return s, b


def _lhsT_taps(w_int):
    """[oc, ic, 3, 3] -> [ic, 9*oc] stationary-operand layout (tap-major)."""
    t = np.transpose(w_int, (2, 3, 1, 0)).reshape(9, C, C)   # [tap, ic, oc]
    return np.transpose(t, (1, 0, 2)).reshape(C, 9 * C)


# ---------------------------------------------------------------- bass build
def _split_multiwaits(nc, mybir):
    """Walrus in this toolchain encodes at most ONE sync wait per instruction.

    Tile emits instructions with several on_wait entries; hoist all but one
    onto same-engine NoOps placed immediately before the instruction."""
    nid = 0
    for fn in nc.m.functions:
        for blk in fn.blocks:
            out = []
            changed = False
            for ins in blk.instructions:
                si = ins.sync_info
                if si is not None and len(si.on_wait) > 1:
                    waits = list(si.on_wait)
                    for w in waits[:-1]:
                        nid += 1
                        nop = mybir.InstNoOp(name=f"I-wfix-{nid}",
                                             engine=ins.engine)
                        nop.sync_info = mybir.SyncInfo(on_wait=[w],
                                                       on_update=[])
                        out.append(nop)
                    ins.sync_info = mybir.SyncInfo(
                        on_wait=[waits[-1]], on_update=list(si.on_update))
                    changed = True
                out.append(ins)
            if changed:
                blk.instructions = out


def _build_module(apply_wfix=True):
    import concourse.bass as bass
    import concourse.mybir as mybir
    import concourse.tile as tile
    from contextlib import ExitStack

    f32 = mybir.dt.float32
    AF = mybir.ActivationFunctionType
    OP = mybir.AluOpType

    nc = bass.Bass("TRN2", target_bir_lowering=False, debug=False,
                   num_devices=N_CORES)

    f32r = mybir.dt.float32r
    f8 = mybir.dt.float8e4
    w1dt = {"f8": f8, "bf16": mybir.dt.bfloat16, "f32r": f32r}[W1DT]

    f16 = mybir.dt.float16
    xdt = f16 if XDT == "f16" else f32r
    x_d = nc.dram_tensor("x15", [BPC, C, H, W], xdt, kind="ExternalInput")
    w2p_d = nc.dram_tensor("w2p", [C, 9 * C], f8, kind="ExternalInput")
    w1r_d = nc.dram_tensor("w1r", [C, 9 * C], w1dt, kind="ExternalInput")
    # columns: [sc1, bi1, sc2, bi2]
    bn_d = nc.dram_tensor("bnv", [C, 4], f32, kind="ExternalInput")
    out_d = nc.dram_tensor("out", [BPC, C, H, W], f32, kind="ExternalOutput")

    with tile.TileContext(nc) as tc, ExitStack() as ctx:
        const = ctx.enter_context(tc.tile_pool(name="const", bufs=1))
        sb = ctx.enter_context(tc.tile_pool(name="sb", bufs=2))
        xp = ctx.enter_context(tc.tile_pool(name="xp", bufs=3))
        ps = ctx.enter_context(tc.tile_pool(name="ps", bufs=4, space="PSUM"))

        warm_w = const.tile([C, C], f32r)
        warm_x = const.tile([C, FREE], f32r)
        w1r_sb = const.tile([C, 9 * C], w1dt)
        bn_sb = const.tile([C, 4], f32)
        w2p_sb = const.tile([C, 9 * C], f8)
        sc1_sb = bn_sb[:, 0:1]
        bi1_sb = bn_sb[:, 1:2]
        sc2_sb = bn_sb[:, 2:3]
        bi2_sb = bn_sb[:, 3:4]

        def emit_x_load(n, first=False):
            """Allocate x tile for image n, DMA + pad-insert (borders are
            memset only for the first ring generation of each slot; interior
            writes never touch them)."""
            x = xp.tile([C, BUF], f16 if XDT == "f16" else f32,
                        tag="x", name=f"x_{n}")
            xw = x if XDT == "f16" else x.bitcast(f32r)
            xwr = xw[:, XB:XB + IMG].rearrange("p (h w) -> p h w", w=WP)
            if n < 3:
                xr0 = x[:, XB:XB + IMG].rearrange("p (h w) -> p h w", w=WP)
                nc.gpsimd.memset(x[:, 0:XB + WP + 1], 0.0)
                nc.gpsimd.memset(x[:, XB + (HPAD - 1) * WP:BUF], 0.0)
                nc.gpsimd.memset(xr0[:, 1:57, 0], 0.0)
                nc.gpsimd.memset(xr0[:, 1:57, 57], 0.0)
            if first:
                # const loads ride the Scalar engine's DGE ring so they
                # stream in parallel with the x pieces on the Sync ring
                for b in range(3):
                    nc.scalar.dma_start(
                        w1r_sb[:, b * 3 * C:(b + 1) * 3 * C],
                        w1r_d.ap()[:, b * 3 * C:(b + 1) * 3 * C])
                nc.scalar.dma_start(bn_sb[:], bn_d.ap())
                nc.scalar.dma_start(w2p_sb[:], w2p_d.ap())
            xs = sb.tile([C, H * W], xdt, tag="xs", name=f"xs_{n}")
            xsr = xs.rearrange("p (h w) -> p h w", w=W)
            xd_flat = x_d.ap()[n].rearrange("p h w -> p (h w)")
            npiece = 8 if first else 4
            rows = H // npiece
            for q in range(npiece):
                r0, r1 = 1 + rows * q, 1 + rows * (q + 1)
                nc.sync.dma_start(xs[:, (r0 - 1) * W:(r1 - 1) * W],
                                  xd_flat[:, (r0 - 1) * W:(r1 - 1) * W])
                nc.vector.tensor_copy(
                    xwr[:, r0:r1, 1:57],
                    xsr[:, r0 - 1:r1 - 1, :])
            return x

        def conv1_rhs(x, cch, dy, dx):
            r0 = 1 + RPC * cch
            off = XB + (r0 + dy) * WP + 1 + dx
            mv = bass.AP(tensor=x.tensor, offset=off,
                         ap=[[BUF, C], [WP, RPC], [1, W]])
            return mv if XDT == "f16" else mv.bitcast(f32r)

        def conv1_affine(v1, cch, p1):
            r0 = 1 + RPC * cch
            dst = v1[:, XB + r0 * WP:XB + (r0 + RPC) * WP].rearrange(
                "p (h w) -> p h w", w=WP)[:, :, 1:57]
            nc.scalar.activation(dst,
                                 p1.rearrange("p (h w) -> p h w", w=W),
                                 AF.Identity, bias=bi1_sb, scale=sc1_sb)

        def emit_conv1_group(n, x, v1, chunks, tap_major=True):
            """Conv1 matmuls + affines over `chunks`.  Tap-major shares the
            stationary operand across the group; chunk-major (image 0) lets
            each chunk start as soon as its x quarter lands."""
            p1s = [ps.tile([C, FREE], f32, tag="p1", name=f"p1_{n}_{c}")
                   for c in chunks]
            order = [(t9, ci) for t9 in range(9) for ci in range(len(chunks))]
            if not tap_major:
                order = [(t9, ci) for ci in range(len(chunks))
                         for t9 in range(9)]
            for t9, ci in order:
                dy, dx = t9 // 3 - 1, t9 % 3 - 1
                nc.tensor.matmul(
                    p1s[ci][:],
                    lhsT=w1r_sb[:, t9 * C:(t9 + 1) * C],
                    rhs=conv1_rhs(x, chunks[ci], dy, dx),
                    start=(t9 == 0), stop=(t9 == 8))
            for ci, cch in enumerate(chunks):
                conv1_affine(v1, cch, p1s[ci])

        def emit_qact(n, v1, act1, half):
            """clip + round for v1 rows ra..rb -> act1 fp8 (both copies)."""
            ra, rb = (1, 33) if half == 0 else (33, 57)
            v1r = v1[:, XB:XB + IMG].rearrange("p (h w) -> p h w", w=WP)
            nc.vector.tensor_scalar(v1r[:, ra:rb, 1:57], v1r[:, ra:rb, 1:57],
                                    0.0, 15.0, op0=OP.max, op1=OP.min)
            ar = act1[:, AB:AB + IMG2].rearrange("p (h w) -> p h w", w=WP2)
            nc.vector.tensor_scalar(ar[:, ra:rb, 1:57], v1r[:, ra:rb, 1:57],
                                    MAGIC, MAGIC, op0=OP.add, op1=OP.subtract)
            ar2 = act1[:, ACT_D + AB:ACT_D + AB + IMG2].rearrange(
                "p (h w) -> p h w", w=WP2)
            nc.vector.tensor_scalar(ar2[:, ra:rb, 1:57], v1r[:, ra:rb, 1:57],
                                    MAGIC, MAGIC, op0=OP.add, op1=OP.subtract)

        def new_act1(n):
            """Allocate the act1 tile; zero its static border region for the
            first ring generation of each slot."""
            act1 = sb.tile([C, ABUF], f8, tag="act1", name=f"act1_{n}")
            if n < 2:
                ar = act1[:, AB:AB + IMG2].rearrange(
                    "p (h w) -> p h w", w=WP2)
                ar2 = act1[:, ACT_D + AB:ACT_D + AB + IMG2].rearrange(
                    "p (h w) -> p h w", w=WP2)
                nc.gpsimd.memset(act1[:, 0:AB + WP2 + 1], 0.0)
                nc.gpsimd.memset(act1[:, AB + (HPAD - 1) * WP2:BUF2], 0.0)
                nc.gpsimd.memset(ar[:, 1:57, 0], 0.0)
                nc.gpsimd.memset(ar[:, 1:57, 57:64], 0.0)
                nc.gpsimd.memset(act1[:, ACT_D:ACT_D + AB + WP2 + 1], 0.0)
                nc.gpsimd.memset(act1[:, ACT_D + AB + (HPAD - 1) * WP2:ABUF],
                                 0.0)
                nc.gpsimd.memset(ar2[:, 1:57, 0], 0.0)
                nc.gpsimd.memset(ar2[:, 1:57, 57:64], 0.0)
            return act1

        def emit_conv2_group(n, x, act1, v2, ost, chunks, quarters):
            """Weight-set-major conv2 over `chunks`, then residual+qact
            quarters (56-wide streams throughout)."""
            inv15 = float(np.float32(1.0) / np.float32(15.0))
            vr = v2[:, XB:XB + IMG2].rearrange("p (h w) -> p h w", w=WP2)
            xr = x[:, XB:XB + IMG].rearrange("p (h w) -> p h w", w=WP)
            ostr = ost.rearrange("p (h w) -> p h w", w=W)
            od_flat = out_d.ap()[n].rearrange("p h w -> p (h w)")

            p2s = [ps.tile([C, FREE], f32, tag="p2", name=f"p2_{n}_{c}")
                   for c in chunks]
            for mi in range(5):
                for ci, cch in enumerate(chunks):
                    r0 = 1 + RPC * cch
                    if mi < 3:
                        dx = mi - 1
                        off_a = AB + (r0 - 1) * WP2 + 1 + dx
                        mv = bass.AP(tensor=act1.tensor, offset=off_a,
                                     ap=[[ABUF, C], [2 * WP2, 2], [WP2, RPC],
                                         [1, W]])
                        wpair = w2p_sb[:, mi * 2 * C:(mi + 1) * 2 * C]\
                            .rearrange("p (two m) -> p two m", two=2)
                        nc.tensor.matmul(
                            p2s[ci][:], lhsT=wpair, rhs=mv,
                            perf_mode=mybir.MatmulPerfMode.DoubleRow,
                            start=(mi == 0), stop=False)
                    elif mi == 3:
                        off_a = AB + r0 * WP2
                        mv = bass.AP(tensor=act1.tensor, offset=off_a,
                                     ap=[[ABUF, C], [ACT_D + 2, 2],
                                         [WP2, RPC], [1, W]])
                        wpair = w2p_sb[:, 6 * C:8 * C].rearrange(
                            "p (two m) -> p two m", two=2)
                        nc.tensor.matmul(
                            p2s[ci][:], lhsT=wpair, rhs=mv,
                            perf_mode=mybir.MatmulPerfMode.DoubleRow,
                            start=False, stop=False)
                    else:
                        off = AB + r0 * WP2 + 1
                        mv = bass.AP(tensor=act1.tensor, offset=off,
                                     ap=[[ABUF, C], [WP2, RPC], [1, W]])
                        nc.tensor.matmul(p2s[ci][:],
                                         lhsT=w2p_sb[:, 8 * C:9 * C],
                                         rhs=mv, start=False, stop=True)
            for ci, cch in enumerate(chunks):
                r0 = 1 + RPC * cch
                dst = v2[:, XB + r0 * WP2:XB + (r0 + RPC) * WP2].rearrange(
                    "p (h w) -> p h w", w=WP2)[:, :, 1:57]
                nc.scalar.activation(dst,
                                     p2s[ci].rearrange("p (h w) -> p h w",
                                                       w=W),
                                     AF.Identity, bias=bi2_sb, scale=sc2_sb)

            for q in quarters:
                # residual + qact + /15 + out-DMA for output rows 14q..14q+13,
                # in 7-row pieces so the gpsimd/scalar/DVE chain pipelines
                for hh in range(2):
                    r0, r1 = 1 + 14 * q + 7 * hh, 8 + 14 * q + 7 * hh
                    vq = vr[:, r0:r1, 1:57]
                    nc.gpsimd.tensor_add(vq, vq, xr[:, r0:r1, 1:57])
                    nc.scalar.activation(vq, vq, AF.Relu)
                    nc.vector.tensor_scalar(vq, vq, 15.0, MAGIC,
                                            op0=OP.min, op1=OP.add)
                    nc.vector.tensor_scalar(ostr[:, r0 - 1:r1 - 1, :],
                                            vr[:, r0:r1, 1:57], MAGIC, inv15,
                                            op0=OP.subtract, op1=OP.mult)
                    nc.sync.dma_start(od_flat[:, (r0 - 1) * W:(r1 - 1) * W],
                                      ost[:, (r0 - 1) * W:(r1 - 1) * W])

        # ------------------------------------------------------- pipeline
        NWARM = int(os.environ.get("K_NWARM", "6"))
        if NWARM:
            nc.gpsimd.memset(warm_w.bitcast(f32)[:], 1.0)
            nc.gpsimd.memset(warm_x.bitcast(f32)[:], 1.0)
            pwm = ps.tile([C, FREE], f32, tag="p1", name="p_warm")
            for _ in range(NWARM):
                nc.tensor.matmul(pwm[:], lhsT=warm_w[:], rhs=warm_x[:],
                                 start=True, stop=True)
        img_v2 = {}

        def conv2_part(n, x_im, act1_im, gi):
            if n not in img_v2:
                img_v2[n] = (sb.tile([C, BUF2], f32, tag="v2",
                                     name=f"v2_{n}"),
                             sb.tile([C, H * W], f32, tag="ost",
                                     name=f"ost_{n}"))
            v2, ost = img_v2[n]
            emit_conv2_group(n, x_im, act1_im, v2, ost, G2[gi], QUARTERS[gi])

        x0 = emit_x_load(0, first=True)
        v1_0 = sb.tile([C, BUF], f32, tag="v1", name="v1_0")
        act1_0 = new_act1(0)
        emit_conv1_group(0, x0, v1_0, (0, 1))
        emit_conv1_group(0, x0, v1_0, (2, 3))
        emit_qact(0, v1_0, act1_0, 0)
        emit_conv1_group(0, x0, v1_0, (4, 5, 6))
        emit_qact(0, v1_0, act1_0, 1)
        conv2_part(0, x0, act1_0, 0)
        x_cur, act1_cur = x0, act1_0

        for n in range(BPC):
            x_nxt = v1_nxt = act1_nxt = None
            if n + 1 < BPC:
                x_nxt = emit_x_load(n + 1)
                v1_nxt = sb.tile([C, BUF], f32, tag="v1", name=f"v1_{n + 1}")
                act1_nxt = new_act1(n + 1)
                emit_conv1_group(n + 1, x_nxt, v1_nxt, G1[0])
                emit_qact(n + 1, v1_nxt, act1_nxt, 0)

            conv2_part(n, x_cur, act1_cur, 1)
            conv2_part(n, x_cur, act1_cur, 2)

            if n + 1 < BPC:
                emit_conv1_group(n + 1, x_nxt, v1_nxt, G1[1])
                emit_qact(n + 1, v1_nxt, act1_nxt, 1)

            conv2_part(n, x_cur, act1_cur, 3)
            if n + 1 < BPC:
                # pull image n+1's first conv2 group into this round so the
                # final round only drains three groups
                conv2_part(n + 1, x_nxt, act1_nxt, 0)

            x_cur, act1_cur = x_nxt, act1_nxt

    if apply_wfix:
        _split_multiwaits(nc, mybir)
    return nc


def _get_module(apply_wfix=True):
    key = ("nc", apply_wfix, W1DT, XDT)
    if key not in _CACHE:
        _CACHE[key] = _build_module(apply_wfix)
    return _CACHE[key]


# ---------------------------------------------------------------- host entry
def _make_in_maps(x, w1, w2, gamma1, beta1, mean1, var1,
                  gamma2, beta2, mean2, var2):
    x15 = np.float32(15.0) * np.asarray(x, np.float32)
    if XDT == "f16":
        x15 = x15.astype(np.float16)
    x15 = x15.reshape(N_CORES, BPC, C, H, W)

    w1i = _quant_weight_int(np.asarray(w1, np.float32))
    w2i = _quant_weight_int(np.asarray(w2, np.float32))
    w2t = _lhsT_taps(w2i)  # [C, 9*C], tap-major (t9 = (dy+1)*3 + dx+1)
    tap = lambda t9: w2t[:, t9 * C:(t9 + 1) * C]
    blocks = []
    for dxi in range(3):           # DR pairs: (dy=-1,dx) then (dy=+1,dx)
        blocks += [tap(dxi), tap(6 + dxi)]
    blocks += [tap(3), tap(5)]     # DR pair: (dy=0,dx=-1) + (dy=0,dx=+1)
    blocks.append(tap(4))          # single: (dy=0,dx=0)
    w2p = np.concatenate(blocks, axis=1).astype(ml_dtypes.float8_e4m3)

    s1, b1 = _bn_affine(np.asarray(gamma1, np.float32), np.asarray(beta1, np.float32),
                        np.asarray(mean1, np.float32), np.asarray(var1, np.float32))
    s2, b2 = _bn_affine(np.asarray(gamma2, np.float32), np.asarray(beta2, np.float32),
                        np.asarray(mean2, np.float32), np.asarray(var2, np.float32))
    # conv PSUM holds 225*conv (15x-or-15a input, 15w weights) -> want 15*bn:
    bnv = np.stack([s1 / 15.0, 15.0 * b1, s2 / 15.0, 15.0 * b2],
                   axis=1).astype(np.float32)  # [C, 4]

    w1np = {"f8": ml_dtypes.float8_e4m3, "bf16": ml_dtypes.bfloat16,
            "f32r": np.float32}[W1DT]
    w1r = _lhsT_taps(w1i).astype(w1np)
    shared = {"w2p": w2p, "w1r": w1r, "bnv": bnv}
    return [{"x15": np.ascontiguousarray(x15[i]), **shared}
            for i in range(N_CORES)]


def kernel(**inputs):
    from concourse.bass_utils import run_bass_kernel_spmd

    nc = _get_module()
    in_maps = _make_in_maps(**inputs)
    res = run_bass_kernel_spmd(nc, in_maps, core_ids=list(range(N_CORES)))
    _CACHE["last_res"] = res
    out = np.concatenate([np.asarray(r["out"], np.float32)
                          for r in res.results], axis=0)
    return out.reshape(B, C, H, W)


# revision 16
# speedup vs baseline: 1.0093x; 1.0017x over previous
"""Trainium2 Bass kernel for a DoReFa-quantized ResNet BasicBlock (inference).

Reference computation (all fp32):
    out = qact(bn2(conv3x3(qact(bn1(conv3x3(x, qw(w1)))), qw(w2))) + x)
with qw = 4-bit DoReFa weight quant, qact = 4-bit activation quant,
x: (64, 128, 56, 56), convs 128->128 stride 1 pad 1.

Sharding: data-parallel over the batch dim, 8 images per NeuronCore on 8 cores.

Per-core kernel design:
  * NCHW with C=128 on SBUF partitions, flattened zero-padded image rows in
    the free dim; a 3x3 conv = shifted 128x128 matmuls accumulated in PSUM
    (8-row chunks, one PSUM bank each).
  * Dual row pitches: conv1/x use 58-wide rows; act1/conv2 use 64-wide rows
    so the fp8 DoubleRow pair stride (2*64 = 128 B) satisfies the %16 rule.
  * Conv2 is bit-exact integer arithmetic in 5 fp8 matmuls per chunk
    (3 DoubleRow dy-pairs, 1 DoubleRow (dy=0,dx=-1)+(dx=+1) pair against a
    16-byte-aligned shifted duplicate of act1, 1 normal center tap).
  * Conv1 streams x in the PE's float32r mode at 1 col/cycle (~207ns/MM:
    walrus reloads the 186ns f32r LDWEIGHTS before every matmul, and no
    narrower weight dtype may pair with an f32r moving operand).
  * Tap-major matmul emission over chunk groups (conv1 {0..3}/{4..6}, conv2
    {0..2}/{3..6}); 4+4 PSUM banks.
  * qact is split into row halves emitted right after each conv1 group;
    conv2 runs as 4 sub-groups with one residual quarter (two 7-row
    pieces: gpsimd add, ScalarE relu, DVE clip-round/scale, DMA out)
    after each, and image n+1's first conv2 group is pulled into round n
    so the final round drains quickly.
  * Six scratch matmuls queued ahead of the pipeline keep the PE busy
    through the head DMA wait so the HAM clock gate opens (2.4GHz)
    before the real matmul stream starts.
  * Pad borders in the x and act1 tiles are written only for the first ring
    generation of each tile slot (all interior writes are border-disjoint),
    removing ~12 GpSimd memsets per image.
  * BN folds to a per-channel affine applied by ScalarE out of PSUM; DoReFa
    staircase = tensor_scalar clip (max,min) + round-half-even via the +2^23
    fp32 trick on VectorE (bit-matches jnp.round).
  * All HBM transfers contiguous (staging tiles + VectorE pad insert); image
    0 is chunk-major, paced by its quarter DMAs, with w1 tap-block DMAs
    interleaved behind the x quarters.
  * A post-Tile pass splits multi-semaphore waits onto same-engine NoOps
    (this walrus encodes at most one sync wait per instruction).
"""

import os
import sys

import numpy as np

for _p in ("/opt/trn_rl_repo", "/opt/pypackages"):
    if _p not in sys.path and os.path.isdir(_p):
        sys.path.insert(0, _p)

import ml_dtypes  # noqa: E402

# ---------------------------------------------------------------- constants
B, C, H, W = 64, 128, 56, 56
N_CORES = 8
BPC = B // N_CORES          # images per core
WP = W + 2                  # conv1/x padded row length (58)
WP2 = 64                    # conv2/act1 padded row length (58 used + 6 dead)
HPAD = H + 2                # padded rows        (58)
IMG = WP * HPAD             # x-layout padded image elems (3364)
IMG2 = WP2 * HPAD           # act1-layout padded image elems (3712)
BUF = IMG + 4               # x/v1 buffer
BUF2 = IMG2 + 4             # act1/v2 buffer
ACT_D = 3726                # shifted act1 copy offset; pair step D+2 %16==0
ABUF = ACT_D + BUF2         # act1 tile width (original + shifted copy)
XB = 1                      # x / v / out buffers: image base offset
AB = 2                      # act1 buffer: base offset
RPC = 8                     # padded rows per PSUM chunk
NCHUNK = H // RPC           # 7 chunks cover output rows 1..56
FREE = RPC * W              # 448 free elems per conv matmul (56-wide)
MAGIC = float(2**23)        # fp32 round-to-nearest-even magic constant
EPS = 1e-5

G1 = ((0, 1, 2, 3), (4, 5, 6))   # conv1 tap-major chunk groups
G2 = ((0, 1), (2, 3), (4, 5), (6,))   # conv2 sub-groups
QUARTERS = ((0,), (1,), (2,), (3,))    # one output quarter after each

# conv1 stationary-weight storage: "f8" | "bf16" | "f32r" (values are exact
# odd ints in [-15,15] in all three; narrower dtypes load faster)
W1DT = os.environ.get("K_W1DT", "f8")
XDT = os.environ.get("K_XDT", "f32r")
if XDT == "f32r":
    W1DT = "f32r"

_CACHE = {}


# ---------------------------------------------------------------- host math
def _quant_weight_int(w):
    """Return 15*quantize_weight(w, 4) which is an exact odd integer in
    [-15, 15], as float32."""
    wt = np.tanh(w.astype(np.float64)).astype(np.float32)
    m = np.float32(np.abs(wt).max())
    wtn = wt / (np.float32(2.0) * m) + np.float32(0.5)      # [0, 1]
    q = np.round(wtn * np.float32(15.0)).astype(np.float32)  # {0..15}, half-even
    return np.float32(2.0) * q - np.float32(15.0)            # odd ints [-15,15]


def _bn_affine(gamma, beta, mean, var):
    """Per-channel (scale, bias) with bn(y) = scale*y + bias, in f64."""
    inv = 1.0 / np.sqrt(var.astype(np.float64) + EPS)
    s = gamma.astype(np.float64) * inv
    b = beta.astype(np.float64) - mean.astype(np.float64) * s
    return s, b


def _lhsT_taps(w_int):
    """[oc, ic, 3, 3] -> [ic, 9*oc] stationary-operand layout (tap-major)."""
    t = np.transpose(w_int, (2, 3, 1, 0)).reshape(9, C, C)   # [tap, ic, oc]
    return np.transpose(t, (1, 0, 2)).reshape(C, 9 * C)


# ---------------------------------------------------------------- bass build
def _split_multiwaits(nc, mybir):
    """Walrus in this toolchain encodes at most ONE sync wait per instruction.

    Tile emits instructions with several on_wait entries; hoist all but one
    onto same-engine NoOps placed immediately before the instruction."""
    nid = 0
    for fn in nc.m.functions:
        for blk in fn.blocks:
            out = []
            changed = False
            for ins in blk.instructions:
                si = ins.sync_info
                if si is not None and len(si.on_wait) > 1:
                    waits = list(si.on_wait)
                    for w in waits[:-1]:
                        nid += 1
                        nop = mybir.InstNoOp(name=f"I-wfix-{nid}",
                                             engine=ins.engine)
                        nop.sync_info = mybir.SyncInfo(on_wait=[w],
                                                       on_update=[])
                        out.append(nop)
                    ins.sync_info = mybir.SyncInfo(
                        on_wait=[waits[-1]], on_update=list(si.on_update))
                    changed = True
                out.append(ins)
            if changed:
                blk.instructions = out


def _build_module(apply_wfix=True):
    import concourse.bass as bass
    import concourse.mybir as mybir
    import concourse.tile as tile
    from contextlib import ExitStack

    f32 = mybir.dt.float32
    AF = mybir.ActivationFunctionType
    OP = mybir.AluOpType

    nc = bass.Bass("TRN2", target_bir_lowering=False, debug=False,
                   num_devices=N_CORES)

    f32r = mybir.dt.float32r
    f8 = mybir.dt.float8e4
    w1dt = {"f8": f8, "bf16": mybir.dt.bfloat16, "f32r": f32r}[W1DT]

    f16 = mybir.dt.float16
    xdt = f16 if XDT == "f16" else f32r
    x_d = nc.dram_tensor("x15", [BPC, C, H, W], xdt, kind="ExternalInput")
    w2p_d = nc.dram_tensor("w2p", [C, 9 * C], f8, kind="ExternalInput")
    w1r_d = nc.dram_tensor("w1r", [C, 9 * C], w1dt, kind="ExternalInput")
    # columns: [sc1, bi1, sc2, bi2]
    bn_d = nc.dram_tensor("bnv", [C, 4], f32, kind="ExternalInput")
    out_d = nc.dram_tensor("out", [BPC, C, H, W], f32, kind="ExternalOutput")

    with tile.TileContext(nc) as tc, ExitStack() as ctx:
        const = ctx.enter_context(tc.tile_pool(name="const", bufs=1))
        sb = ctx.enter_context(tc.tile_pool(name="sb", bufs=2))
        xp = ctx.enter_context(tc.tile_pool(name="xp", bufs=3))
        ps = ctx.enter_context(tc.tile_pool(name="ps", bufs=4, space="PSUM"))

        warm_w = const.tile([C, C], f32r)
        warm_x = const.tile([C, FREE], f32r)
        w1r_sb = const.tile([C, 9 * C], w1dt)
        bn_sb = const.tile([C, 4], f32)
        w2p_sb = const.tile([C, 9 * C], f8)
        sc1_sb = bn_sb[:, 0:1]
        bi1_sb = bn_sb[:, 1:2]
        sc2_sb = bn_sb[:, 2:3]
        bi2_sb = bn_sb[:, 3:4]

        def emit_x_load(n, first=False):
            """Allocate x tile for image n, DMA + pad-insert (borders are
            memset only for the first ring generation of each slot; interior
            writes never touch them)."""
            x = xp.tile([C, BUF], f16 if XDT == "f16" else f32,
                        tag="x", name=f"x_{n}")
            xw = x if XDT == "f16" else x.bitcast(f32r)
            xwr = xw[:, XB:XB + IMG].rearrange("p (h w) -> p h w", w=WP)
            if n < 3:
                xr0 = x[:, XB:XB + IMG].rearrange("p (h w) -> p h w", w=WP)
                nc.gpsimd.memset(x[:, 0:XB + WP + 1], 0.0)
                nc.gpsimd.memset(x[:, XB + (HPAD - 1) * WP:BUF], 0.0)
                nc.gpsimd.memset(xr0[:, 1:57, 0], 0.0)
                nc.gpsimd.memset(xr0[:, 1:57, 57], 0.0)
            if first:
                # const loads ride the Scalar engine's DGE ring so they
                # stream in parallel with the x pieces on the Sync ring
                for b in range(3):
                    nc.scalar.dma_start(
                        w1r_sb[:, b * 3 * C:(b + 1) * 3 * C],
                        w1r_d.ap()[:, b * 3 * C:(b + 1) * 3 * C])
                nc.scalar.dma_start(bn_sb[:], bn_d.ap())
                nc.scalar.dma_start(w2p_sb[:], w2p_d.ap())
            xs = sb.tile([C, H * W], xdt, tag="xs", name=f"xs_{n}")
            xsr = xs.rearrange("p (h w) -> p h w", w=W)
            xd_flat = x_d.ap()[n].rearrange("p h w -> p (h w)")
            npiece = 8 if first else 4
            rows = H // npiece
            for q in range(npiece):
                r0, r1 = 1 + rows * q, 1 + rows * (q + 1)
                nc.sync.dma_start(xs[:, (r0 - 1) * W:(r1 - 1) * W],
                                  xd_flat[:, (r0 - 1) * W:(r1 - 1) * W])
                nc.vector.tensor_copy(
                    xwr[:, r0:r1, 1:57],
                    xsr[:, r0 - 1:r1 - 1, :])
            return x

        def conv1_rhs(x, cch, dy, dx):
            r0 = 1 + RPC * cch
            off = XB + (r0 + dy) * WP + 1 + dx
            mv = bass.AP(tensor=x.tensor, offset=off,
                         ap=[[BUF, C], [WP, RPC], [1, W]])
            return mv if XDT == "f16" else mv.bitcast(f32r)

        def conv1_affine(v1, cch, p1):
            r0 = 1 + RPC * cch
            dst = v1[:, XB + r0 * WP:XB + (r0 + RPC) * WP].rearrange(
                "p (h w) -> p h w", w=WP)[:, :, 1:57]
            nc.scalar.activation(dst,
                                 p1.rearrange("p (h w) -> p h w", w=W),
                                 AF.Identity, bias=bi1_sb, scale=sc1_sb)

        def emit_conv1_group(n, x, v1, chunks, tap_major=True):
            """Conv1 matmuls + affines over `chunks`.  Tap-major shares the
            stationary operand across the group; chunk-major (image 0) lets
            each chunk start as soon as its x quarter lands."""
            p1s = [ps.tile([C, FREE], f32, tag="p1", name=f"p1_{n}_{c}")
                   for c in chunks]
            order = [(t9, ci) for t9 in range(9) for ci in range(len(chunks))]
            if not tap_major:
                order = [(t9, ci) for ci in range(len(chunks))
                         for t9 in range(9)]
            for t9, ci in order:
                dy, dx = t9 // 3 - 1, t9 % 3 - 1
                nc.tensor.matmul(
                    p1s[ci][:],
                    lhsT=w1r_sb[:, t9 * C:(t9 + 1) * C],
                    rhs=conv1_rhs(x, chunks[ci], dy, dx),
                    start=(t9 == 0), stop=(t9 == 8))
            for ci, cch in enumerate(chunks):
                conv1_affine(v1, cch, p1s[ci])

        def emit_qact(n, v1, act1, half):
            """clip + round for v1 rows ra..rb -> act1 fp8 (both copies)."""
            ra, rb = (1, 33) if half == 0 else (33, 57)
            v1r = v1[:, XB:XB + IMG].rearrange("p (h w) -> p h w", w=WP)
            nc.vector.tensor_scalar(v1r[:, ra:rb, 1:57], v1r[:, ra:rb, 1:57],
                                    0.0, 15.0, op0=OP.max, op1=OP.min)
            ar = act1[:, AB:AB + IMG2].rearrange("p (h w) -> p h w", w=WP2)
            nc.vector.tensor_scalar(ar[:, ra:rb, 1:57], v1r[:, ra:rb, 1:57],
                                    MAGIC, MAGIC, op0=OP.add, op1=OP.subtract)
            ar2 = act1[:, ACT_D + AB:ACT_D + AB + IMG2].rearrange(
                "p (h w) -> p h w", w=WP2)
            nc.vector.tensor_scalar(ar2[:, ra:rb, 1:57], v1r[:, ra:rb, 1:57],
                                    MAGIC, MAGIC, op0=OP.add, op1=OP.subtract)

        def new_act1(n):
            """Allocate the act1 tile; zero its static border region for the
            first ring generation of each slot."""
            act1 = sb.tile([C, ABUF], f8, tag="act1", name=f"act1_{n}")
            if n < 2:
                ar = act1[:, AB:AB + IMG2].rearrange(
                    "p (h w) -> p h w", w=WP2)
                ar2 = act1[:, ACT_D + AB:ACT_D + AB + IMG2].rearrange(
                    "p (h w) -> p h w", w=WP2)
                nc.gpsimd.memset(act1[:, 0:AB + WP2 + 1], 0.0)
                nc.gpsimd.memset(act1[:, AB + (HPAD - 1) * WP2:BUF2], 0.0)
                nc.gpsimd.memset(ar[:, 1:57, 0], 0.0)
                nc.gpsimd.memset(ar[:, 1:57, 57:64], 0.0)
                nc.gpsimd.memset(act1[:, ACT_D:ACT_D + AB + WP2 + 1], 0.0)
                nc.gpsimd.memset(act1[:, ACT_D + AB + (HPAD - 1) * WP2:ABUF],
                                 0.0)
                nc.gpsimd.memset(ar2[:, 1:57, 0], 0.0)
                nc.gpsimd.memset(ar2[:, 1:57, 57:64], 0.0)
            return act1

        def emit_conv2_group(n, x, act1, v2, ost, chunks, quarters):
            """Weight-set-major conv2 over `chunks`, then residual+qact
            quarters (56-wide streams throughout)."""
            inv15 = float(np.float32(1.0) / np.float32(15.0))
            vr = v2[:, XB:XB + IMG2].rearrange("p (h w) -> p h w", w=WP2)
            xr = x[:, XB:XB + IMG].rearrange("p (h w) -> p h w", w=WP)
            ostr = ost.rearrange("p (h w) -> p h w", w=W)
            od_flat = out_d.ap()[n].rearrange("p h w -> p (h w)")

            p2s = [ps.tile([C, FREE], f32, tag="p2", name=f"p2_{n}_{c}")
                   for c in chunks]
            for mi in range(5):
                for ci, cch in enumerate(chunks):
                    r0 = 1 + RPC * cch
                    if mi < 3:
                        dx = mi - 1
                        off_a = AB + (r0 - 1) * WP2 + 1 + dx
                        mv = bass.AP(tensor=act1.tensor, offset=off_a,
                                     ap=[[ABUF, C], [2 * WP2, 2], [WP2, RPC],
                                         [1, W]])
                        wpair = w2p_sb[:, mi * 2 * C:(mi + 1) * 2 * C]\
                            .rearrange("p (two m) -> p two m", two=2)
                        nc.tensor.matmul(
                            p2s[ci][:], lhsT=wpair, rhs=mv,
                            perf_mode=mybir.MatmulPerfMode.DoubleRow,
                            start=(mi == 0), stop=False)
                    elif mi == 3:
                        off_a = AB + r0 * WP2
                        mv = bass.AP(tensor=act1.tensor, offset=off_a,
                                     ap=[[ABUF, C], [ACT_D + 2, 2],
                                         [WP2, RPC], [1, W]])
                        wpair = w2p_sb[:, 6 * C:8 * C].rearrange(
                            "p (two m) -> p two m", two=2)
                        nc.tensor.matmul(
                            p2s[ci][:], lhsT=wpair, rhs=mv,
                            perf_mode=mybir.MatmulPerfMode.DoubleRow,
                            start=False, stop=False)
                    else:
                        off = AB + r0 * WP2 + 1
                        mv = bass.AP(tensor=act1.tensor, offset=off,
                                     ap=[[ABUF, C], [WP2, RPC], [1, W]])
                        nc.tensor.matmul(p2s[ci][:],
                                         lhsT=w2p_sb[:, 8 * C:9 * C],
                                         rhs=mv, start=False, stop=True)
            for ci, cch in enumerate(chunks):
                r0 = 1 + RPC * cch
                dst = v2[:, XB + r0 * WP2:XB + (r0 + RPC) * WP2].rearrange(
                    "p (h w) -> p h w", w=WP2)[:, :, 1:57]
                nc.scalar.activation(dst,
                                     p2s[ci].rearrange("p (h w) -> p h w",
                                                       w=W),
                                     AF.Identity, bias=bi2_sb, scale=sc2_sb)

            for q in quarters:
                # residual + qact + /15 + out-DMA for output rows 14q..14q+13,
                # in 7-row pieces so the gpsimd/scalar/DVE chain pipelines
                for hh in range(2):
                    r0, r1 = 1 + 14 * q + 7 * hh, 8 + 14 * q + 7 * hh
                    vq = vr[:, r0:r1, 1:57]
                    nc.gpsimd.tensor_add(vq, vq, xr[:, r0:r1, 1:57])
                    # round(max(u,0)) then clip top; ost holds the exact int
                    # q in [0,15] -- the final /15 happens on the host (the
                    # same single fp32 multiply the device op1 would do)
                    nc.vector.tensor_scalar(vq, vq, 0.0, MAGIC,
                                            op0=OP.max, op1=OP.add)
                    nc.vector.tensor_scalar(ostr[:, r0 - 1:r1 - 1, :],
                                            vr[:, r0:r1, 1:57],
                                            MAGIC + 15.0, MAGIC,
                                            op0=OP.min, op1=OP.subtract)
                    nc.sync.dma_start(od_flat[:, (r0 - 1) * W:(r1 - 1) * W],
                                      ost[:, (r0 - 1) * W:(r1 - 1) * W])

        # ------------------------------------------------------- pipeline
        NWARM = int(os.environ.get("K_NWARM", "6"))
        if NWARM:
            nc.gpsimd.memset(warm_w.bitcast(f32)[:], 1.0)
            nc.gpsimd.memset(warm_x.bitcast(f32)[:], 1.0)
            pwm = ps.tile([C, FREE], f32, tag="p1", name="p_warm")
            for _ in range(NWARM):
                nc.tensor.matmul(pwm[:], lhsT=warm_w[:], rhs=warm_x[:],
                                 start=True, stop=True)
        img_v2 = {}

        def conv2_part(n, x_im, act1_im, gi):
            if n not in img_v2:
                img_v2[n] = (sb.tile([C, BUF2], f32, tag="v2",
                                     name=f"v2_{n}"),
                             sb.tile([C, H * W], f32, tag="ost",
                                     name=f"ost_{n}"))
            v2, ost = img_v2[n]
            emit_conv2_group(n, x_im, act1_im, v2, ost, G2[gi], QUARTERS[gi])

        x0 = emit_x_load(0, first=True)
        v1_0 = sb.tile([C, BUF], f32, tag="v1", name="v1_0")
        act1_0 = new_act1(0)
        emit_conv1_group(0, x0, v1_0, (0, 1))
        emit_conv1_group(0, x0, v1_0, (2, 3))
        emit_qact(0, v1_0, act1_0, 0)
        emit_conv1_group(0, x0, v1_0, (4, 5, 6))
        emit_qact(0, v1_0, act1_0, 1)
        conv2_part(0, x0, act1_0, 0)
        conv2_part(0, x0, act1_0, 1)
        x_cur, act1_cur = x0, act1_0

        for n in range(BPC):
            x_nxt = v1_nxt = act1_nxt = None
            if n + 1 < BPC:
                x_nxt = emit_x_load(n + 1)
                v1_nxt = sb.tile([C, BUF], f32, tag="v1", name=f"v1_{n + 1}")
                act1_nxt = new_act1(n + 1)
                emit_conv1_group(n + 1, x_nxt, v1_nxt, G1[0])
                emit_qact(n + 1, v1_nxt, act1_nxt, 0)

            conv2_part(n, x_cur, act1_cur, 2)

            if n + 1 < BPC:
                emit_conv1_group(n + 1, x_nxt, v1_nxt, G1[1])
                emit_qact(n + 1, v1_nxt, act1_nxt, 1)

            conv2_part(n, x_cur, act1_cur, 3)
            if n + 1 < BPC:
                # pull image n+1's first two conv2 groups into this round so
                # the final round only drains groups 2 and 3
                conv2_part(n + 1, x_nxt, act1_nxt, 0)
                conv2_part(n + 1, x_nxt, act1_nxt, 1)

            x_cur, act1_cur = x_nxt, act1_nxt

    if apply_wfix:
        _split_multiwaits(nc, mybir)
    return nc


def _get_module(apply_wfix=True):
    key = ("nc", apply_wfix, W1DT, XDT)
    if key not in _CACHE:
        _CACHE[key] = _build_module(apply_wfix)
    return _CACHE[key]


# ---------------------------------------------------------------- host entry
def _make_in_maps(x, w1, w2, gamma1, beta1, mean1, var1,
                  gamma2, beta2, mean2, var2):
    x15 = np.float32(15.0) * np.asarray(x, np.float32)
    if XDT == "f16":
        x15 = x15.astype(np.float16)
    x15 = x15.reshape(N_CORES, BPC, C, H, W)

    w1i = _quant_weight_int(np.asarray(w1, np.float32))
    w2i = _quant_weight_int(np.asarray(w2, np.float32))
    w2t = _lhsT_taps(w2i)  # [C, 9*C], tap-major (t9 = (dy+1)*3 + dx+1)
    tap = lambda t9: w2t[:, t9 * C:(t9 + 1) * C]
    blocks = []
    for dxi in range(3):           # DR pairs: (dy=-1,dx) then (dy=+1,dx)
        blocks += [tap(dxi), tap(6 + dxi)]
    blocks += [tap(3), tap(5)]     # DR pair: (dy=0,dx=-1) + (dy=0,dx=+1)
    blocks.append(tap(4))          # single: (dy=0,dx=0)
    w2p = np.concatenate(blocks, axis=1).astype(ml_dtypes.float8_e4m3)

    s1, b1 = _bn_affine(np.asarray(gamma1, np.float32), np.asarray(beta1, np.float32),
                        np.asarray(mean1, np.float32), np.asarray(var1, np.float32))
    s2, b2 = _bn_affine(np.asarray(gamma2, np.float32), np.asarray(beta2, np.float32),
                        np.asarray(mean2, np.float32), np.asarray(var2, np.float32))
    # conv PSUM holds 225*conv (15x-or-15a input, 15w weights) -> want 15*bn:
    bnv = np.stack([s1 / 15.0, 15.0 * b1, s2 / 15.0, 15.0 * b2],
                   axis=1).astype(np.float32)  # [C, 4]

    w1np = {"f8": ml_dtypes.float8_e4m3, "bf16": ml_dtypes.bfloat16,
            "f32r": np.float32}[W1DT]
    w1r = _lhsT_taps(w1i).astype(w1np)
    shared = {"w2p": w2p, "w1r": w1r, "bnv": bnv}
    return [{"x15": np.ascontiguousarray(x15[i]), **shared}
            for i in range(N_CORES)]


def kernel(**inputs):
    from concourse.bass_utils import run_bass_kernel_spmd

    nc = _get_module()
    in_maps = _make_in_maps(**inputs)
    res = run_bass_kernel_spmd(nc, in_maps, core_ids=list(range(N_CORES)))
    _CACHE["last_res"] = res
    out = np.concatenate([np.asarray(r["out"], np.float32)
                          for r in res.results], axis=0)
    out = out * np.float32(1.0 / 15.0)
    return out.reshape(B, C, H, W)
